# revision 13
# baseline (speedup 1.0000x reference)
"""Attention-pooling kernel for Trainium2 (8 NeuronCores, data parallel).

Computes, for full inputs query [B, D], keys [B, L, D], W [1, D]:
    inter  = keys * query[:, None, :]
    scores = tanh(einsum('bld,od->blo', inter, W))
    p      = softmax(scores, axis=1)
    out    = sum(p * keys, axis=1)                      # [B, D]

Sharding: batch dim split evenly across 8 cores; W replicated.

Per-core layout: batch rows on partitions, [128, L*D] tiles. DVE does the
two elementwise multiplies and the segmented reductions; ACT does
tanh/exp (+ softmax denominator via accum_out).
"""

import sys

if "/opt/trn_rl_repo" not in sys.path:
    sys.path.insert(0, "/opt/trn_rl_repo")

import numpy as np

import concourse.bacc as bacc
import concourse.bass as bass
import concourse.mybir as mybir
import concourse.tile as tile
from concourse.bass_utils import run_bass_kernel_spmd

B, L, D = 16384, 200, 64
NCORES = 8
BC = B // NCORES  # batch rows per core
PT = 128          # partition tile (batch rows per SBUF tile)
NT = BC // PT     # tiles per core

# variant = (keys_bf16, d_tree, l_tree[, pe2, rk1, hwcast])
FAST_VARIANT = (True, False, True, True, True)
SAFE_VARIANT = (False, False, False)
DEFAULT_VARIANT = SAFE_VARIANT

_cache = {}
_run_state = {
    "variant": FAST_VARIANT,
    "checked": False,
    "use_v2": True,
    "use_v3": False,
    "checked_v3": False,
    "use_v4": True,
    "checked_v4": False,
}


def _tree_reduce_outer(nc, pool, src_ap, n_outer, inner, dtype, out_ap, tag):
    """Sum over the OUTER axis of a [PT, n_outer, inner] view via halving
    tensor_tensor adds (inner dim stays contiguous, 2x-mode eligible for
    bf16). Final [PT, inner] f32 result lands in out_ap."""
    cur = src_ap
    n = n_outer
    lvl = 0
    while n > 1:
        h, odd = n // 2, n % 2
        if h + odd == 1:
            nc.vector.tensor_add(
                out_ap.unsqueeze(1), cur[:, 0:1, :], cur[:, 1:2, :]
            )
            return
        # ping-pong tags: level k+1 reads level k, so they must coexist
        t = pool.tile([PT, (h + odd) * inner], dtype, tag=f"{tag}{lvl % 2}")
        dst = t[:].rearrange("p (n i) -> p n i", n=h + odd)
        nc.vector.tensor_add(dst[:, 0:h, :], cur[:, 0:h, :], cur[:, h : 2 * h, :])
        if odd:
            nc.vector.tensor_copy(dst[:, h : h + 1, :], cur[:, 2 * h : n, :])
        cur = dst
        n = h + odd
        lvl += 1


def _tree_reduce_inner(nc, pool, src_ap, outer, n_inner, dtype, out_ap, tag):
    """Sum over the INNER axis of a [PT, outer, n_inner] view via halving
    tensor_tensor adds on contiguous inner slices. n_inner must be a power
    of two. Final [PT, outer] f32 result lands in out_ap."""
    cur = src_ap
    n = n_inner
    lvl = 0
    while n > 1:
        h = n // 2
        if h == 1:
            nc.vector.tensor_add(
                out_ap.unsqueeze(2), cur[:, :, 0:1], cur[:, :, 1:2]
            )
            return
        t = pool.tile([PT, outer * h], dtype, tag=f"{tag}{lvl % 2}")
        dst = t[:].rearrange("p (o i) -> p o i", o=outer)
        nc.vector.tensor_add(dst, cur[:, :, 0:h], cur[:, :, h:n])
        cur = dst
        n = h
        lvl += 1


def _build_bass(variant):
    keys_bf16, d_tree, l_tree = variant[:3]
    pe2 = variant[3] if len(variant) > 3 else False
    rk1 = variant[4] if len(variant) > 4 else False
    hwcast = variant[5] if len(variant) > 5 else False  # f32 HWDGE load + DVE convert
    rk4 = variant[6] if len(variant) > 6 else False  # 32x32 block-diag tile_position
    assert not rk1 or keys_bf16, "rank-1 scoring requires bf16 keys"
    f32 = mybir.dt.float32
    bf16 = mybir.dt.bfloat16
    kdt = bf16 if keys_bf16 else f32
    mdt = bf16 if keys_bf16 else f32  # multiply output dtype
    AF = mybir.ActivationFunctionType
    X = mybir.AxisListType.X

    nc = bacc.Bacc("TRN2", target_bir_lowering=False, debug=False, num_devices=NCORES)
    q_h = nc.declare_dram_parameter("query", [BC, D], f32, isOutput=False)
    k_h = nc.declare_dram_parameter("keys", [BC, L, D], f32, isOutput=False)
    w_h = nc.declare_dram_parameter("W", [PT, D], f32, isOutput=False)
    if rk1:
        e_h = nc.declare_dram_parameter(
            "eye", [PT, 32 if rk4 else PT], bf16, isOutput=False
        )
    o_h = nc.declare_dram_parameter("out", [BC, D], f32, isOutput=True)

    with tile.TileContext(nc) as tc:
        with (
            tc.tile_pool(name="keys", bufs=2) as kp,
            tc.tile_pool(name="work", bufs=2) as wp,
            tc.tile_pool(name="tree", bufs=1) as tp,
            tc.tile_pool(name="small", bufs=2) as sp,
            tc.tile_pool(name="diag", bufs=3) as dgp,
            tc.tile_pool(name="psum", bufs=2, space="PSUM") as pp,
            tc.tile_pool(name="const", bufs=1) as cp,
        ):
            if rk1:
                ew = 32 if rk4 else PT
                eye0 = cp.tile([PT, ew], bf16)
                nc.sync.dma_start(eye0[:], e_h[:])
                eye_t = cp.tile([PT, ew], bf16)
                nc.vector.tensor_copy(eye_t[:], eye0[:])
            # W pre-broadcast to all 128 partitions on the host.
            wb0 = cp.tile([PT, D], f32)
            nc.sync.dma_start(wb0[:], w_h[:])
            # Route through a DVE copy so downstream DVE ops depend on it via
            # program order rather than an extra DMA semaphore wait.
            wb = cp.tile([PT, D], f32)
            nc.vector.tensor_copy(wb[:], wb0[:])

            for t in range(NT):
                rows = slice(t * PT, (t + 1) * PT)

                kt = kp.tile(
                    [PT, L * D], kdt, tag="keys",
                    bufs=1 if hwcast else (3 if keys_bf16 else 2),
                )
                if keys_bf16 and hwcast:
                    ktf = kp.tile([PT, L * D], f32, tag="keysf")
                    nc.sync.dma_start(
                        ktf[:], k_h[rows].rearrange("b l d -> b (l d)")
                    )
                    nc.vector.tensor_copy(kt[:], ktf[:])
                elif keys_bf16:
                    # SWDGE cast-DMA: f32 HBM -> bf16 SBUF
                    nc.gpsimd.dma_start(
                        kt[:], k_h[rows].rearrange("b l d -> b (l d)")
                    )
                else:
                    nc.sync.dma_start(
                        kt[:], k_h[rows].rearrange("b l d -> b (l d)")
                    )
                qt = sp.tile([PT, D], f32, tag="q")
                nc.sync.dma_start(qt[:], q_h[rows, :])

                k3 = kt[:].rearrange("p (l d) -> p l d", l=L)

                if rk1:
                    # v = q * W kept f32, then duplicated into adjacent bf16
                    # pairs (v2p[2d], v2p[2d+1]) = v[d] for the paired
                    # broadcast below.
                    vt = sp.tile([PT, D], f32, tag="v")
                    nc.vector.tensor_mul(vt[:], qt[:], wb[:])
                    v2p = sp.tile([PT, 2 * D], bf16, tag="v2p")
                    v2v = v2p[:].rearrange("p (d two) -> p d two", two=2)
                    nc.vector.tensor_copy(v2v[:, :, 0], vt[:])
                    nc.vector.tensor_copy(v2v[:, :, 1], vt[:])
                    # Build all 64 diag(v[:, d]) blocks in one 2x-mode TT:
                    # dg_all[p, d, j] = eye[p, j] * v[p, d]
                    ew = 32 if rk4 else PT
                    dga = dgp.tile([PT, D * ew], bf16, tag="dg", bufs=1 if hwcast else 3)
                    nc.vector.tensor_mul(
                        dga[:].rearrange(
                            "p (d j2 two) -> p d j2 two", d=D, two=2
                        ),
                        eye_t[:]
                        .rearrange("p (j2 two) -> p j2 two", two=2)
                        .unsqueeze(1)
                        .broadcast_to([PT, D, ew // 2, 2]),
                        v2v.unsqueeze(2).broadcast_to([PT, D, ew // 2, 2]),
                    )
                    # scores[b, l] = sum_d v[b, d] * keys[b, l, d] as
                    # accumulating rank-1 diag matmuls on the TensorEngine:
                    # lhsT = diag(v[:, d]), rhs = keys[:, :, d]
                    psc = pp.tile([PT, L], f32, tag="sc")
                    dg3 = dga[:].rearrange("p (d j) -> p d j", d=D)
                    for d in range(D):
                        if rk4:
                            # four concurrent 32x32 diag-block matmuls
                            for i in range(4):
                                s = slice(32 * i, 32 * i + 32)
                                nc.tensor.matmul(
                                    psc[s, :],
                                    dg3[s, d, :],
                                    k3[s, :, d],
                                    start=(d == 0),
                                    stop=(d == D - 1),
                                    tile_position=(32 * i, 32 * i),
                                )
                        else:
                            nc.tensor.matmul(
                                psc[:],
                                dg3[:, d, :],
                                k3[:, :, d],
                                start=(d == 0),
                                stop=(d == D - 1),
                            )
                    scores = psc
                else:
                    # v = q * W  (per-partition [128, 64])
                    vt = sp.tile([PT, D], mdt, tag="v")
                    nc.vector.tensor_mul(vt[:], qt[:], wb[:])

                    # inter = keys * v (v broadcast along l)
                    inter = wp.tile([PT, L * D], mdt, tag="work")
                    i3 = inter[:].rearrange("p (l d) -> p l d", l=L)
                    nc.vector.tensor_mul(
                        i3, k3, vt[:].unsqueeze(1).broadcast_to([PT, L, D])
                    )

                    # scores[b, l] = sum_d inter
                    scores = sp.tile([PT, L], f32, tag="sc")
                    if d_tree:
                        _tree_reduce_inner(nc, tp, i3, L, D, mdt, scores[:], "dtree")
                    else:
                        nc.vector.reduce_sum(scores[:], i3, axis=X)

                # tanh then exp (same ACT table set); accumulate softmax denom
                th = sp.tile([PT, L], f32, tag="th")
                nc.scalar.activation(th[:], scores[:], AF.Tanh)
                S = sp.tile([PT, 1], f32, tag="S")
                wk = wp.tile([PT, L * D], mdt, tag="work")
                w3 = wk[:].rearrange("p (l d) -> p l d", l=L)
                if pe2:
                    # exp weights duplicated into adjacent pairs so the
                    # broadcast-along-d AP has innermost step 1 (4B-aligned
                    # bf16 pair) -> DVE 2x_1P packed mode for the multiply.
                    ped = sp.tile([PT, 2 * L], mdt, tag="pe")
                    p3 = ped[:].rearrange("p (l two) -> p l two", two=2)
                    nc.scalar.activation(p3[:, :, 0], th[:], AF.Exp, accum_out=S[:])
                    nc.scalar.activation(p3[:, :, 1], th[:], AF.Exp)
                    sinv = sp.tile([PT, 1], f32, tag="sinv")
                    nc.vector.reciprocal(sinv[:], S[:])
                    nc.vector.tensor_mul(
                        wk[:].rearrange("p (l d2 two) -> p l d2 two", l=L, two=2),
                        kt[:].rearrange("p (l d2 two) -> p l d2 two", l=L, two=2),
                        p3.unsqueeze(2).broadcast_to([PT, L, D // 2, 2]),
                    )
                else:
                    pe = sp.tile([PT, L], mdt, tag="pe")
                    nc.scalar.activation(pe[:], th[:], AF.Exp, accum_out=S[:])
                    sinv = sp.tile([PT, 1], f32, tag="sinv")
                    nc.vector.reciprocal(sinv[:], S[:])
                    # wk = keys * exp(scores) (broadcast along d)
                    nc.vector.tensor_mul(
                        w3, k3, pe[:].unsqueeze(2).broadcast_to([PT, L, D])
                    )

                # out_unnorm[b, d] = sum_l wk
                ou = sp.tile([PT, D], f32, tag="ou")
                if l_tree:
                    _tree_reduce_outer(nc, tp, w3, L, D, mdt, ou[:], "ltree")
                else:
                    nc.vector.reduce_sum(
                        ou[:],
                        wk[:].rearrange("p (l d) -> p d l", l=L),
                        axis=X,
                    )
                # normalize by softmax denominator
                of = sp.tile([PT, D], f32, tag="of")
                nc.vector.tensor_scalar_mul(of[:], ou[:], sinv[:])
                nc.sync.dma_start(o_h[rows, :], of[:])

    nc.compile()
    return nc


def _get_nc(variant=DEFAULT_VARIANT):
    key = tuple(variant)
    if key not in _cache:
        _cache[key] = _build_bass(key)
    return _cache[key]


def run_sharded(query, keys, W, trace=False, variant=DEFAULT_VARIANT):
    """Run the SPMD kernel; returns (out [B, D], BassKernelResults)."""
    query = np.ascontiguousarray(query, dtype=np.float32)
    keys = np.ascontiguousarray(keys, dtype=np.float32)
    W = np.ascontiguousarray(W, dtype=np.float32)
    nc = _get_nc(variant)
    w_b = np.ascontiguousarray(np.broadcast_to(W.reshape(1, D), (PT, D)))
    extra = {}
    if len(variant) > 4 and variant[4]:
        import ml_dtypes

        if len(variant) > 6 and variant[6]:
            e = np.zeros((PT, 32), dtype=ml_dtypes.bfloat16)
            e[np.arange(PT), np.arange(PT) % 32] = 1
            extra["eye"] = e
        else:
            extra["eye"] = np.eye(PT, dtype=ml_dtypes.bfloat16)
    in_maps = [
        {
            "query": query[i * BC : (i + 1) * BC],
            "keys": keys[i * BC : (i + 1) * BC],
            "W": w_b,
            **extra,
        }
        for i in range(NCORES)
    ]
    res = run_bass_kernel_spmd(nc, in_maps, core_ids=list(range(NCORES)), trace=trace)
    out = np.concatenate([res.results[i]["out"] for i in range(NCORES)], axis=0)
    return out, res


def _spot_check(out, query, keys, W, n=512):
    """Scaled absmax error of a row subset vs a float64 numpy oracle."""
    idx = np.random.default_rng(0).choice(B, n, replace=False)
    q = query[idx].astype(np.float64)
    k = keys[idx].astype(np.float64)
    w = W.reshape(-1).astype(np.float64)
    sc = np.tanh(((k * q[:, None, :]) * w).sum(-1))
    p = np.exp(sc)
    p /= p.sum(1, keepdims=True)
    ref = (p[:, :, None] * k).sum(1)
    return np.abs(out[idx] - ref).max() / max(np.abs(ref).max(), 1e-6)




# ---------------------------------------------------------------------------
# v2: software-pipelined kernel (PE/DVE hybrid scoring, ACT offloads)
# ---------------------------------------------------------------------------

f32 = mybir.dt.float32
bf16 = mybir.dt.bfloat16

V2_CFG = dict(pd=64, pf=2, ew=64)

def _build_v2(cfg_items):
    cfg = dict(cfg_items)
    pd = cfg["pd"]
    PF = cfg.get("pf", 2)
    EW = cfg.get("ew", 32)  # diag block width (PE tile size); 32/64/128
    dd = D - pd  # dims scored on DVE
    assert pd % 2 == 0 and pd > 0
    assert EW in (32, 64, 128)

    AF = mybir.ActivationFunctionType
    nc = bacc.Bacc("TRN2", target_bir_lowering=False, debug=False, num_devices=NCORES)
    q_h = nc.declare_dram_parameter("query", [BC, D], f32, isOutput=False)
    k_h = nc.declare_dram_parameter("keys", [BC, L, D], f32, isOutput=False)
    w_h = nc.declare_dram_parameter("W", [PT, D], f32, isOutput=False)
    e_h = nc.declare_dram_parameter("eye", [PT, EW], bf16, isOutput=False)
    o_h = nc.declare_dram_parameter("out", [BC, D], f32, isOutput=True)

    with tile.TileContext(nc) as tc:
        with (
            tc.tile_pool(name="keys", bufs=PF + 1) as kp,
            tc.tile_pool(name="wk", bufs=1) as wkp,
            tc.tile_pool(name="tree", bufs=1) as tp,
            tc.tile_pool(name="small", bufs=3) as sp,
            tc.tile_pool(name="diag", bufs=3) as dgp,
            tc.tile_pool(name="psum", bufs=2, space="PSUM") as pp,
            tc.tile_pool(name="const", bufs=1) as cp,
        ):
            eye0 = cp.tile([PT, EW], bf16)
            nc.gpsimd.dma_start(eye0[:], e_h[:])
            eye_t = cp.tile([PT, EW], bf16)
            nc.vector.tensor_copy(eye_t[:], eye0[:])
            wb0 = cp.tile([PT, D], f32)
            nc.gpsimd.dma_start(wb0[:], w_h[:])
            wb = cp.tile([PT, D], f32)
            nc.vector.tensor_copy(wb[:], wb0[:])

            kts = {}
            dgas = {}
            vbs = {}
            pscs = {}
            scs = {}
            pexps = {}
            sinvs = {}

            # all query tiles upfront: keeps the steady-state DVE off the
            # HWDGE ring (whose FIFO couples q-loads behind of-stores)
            qall = cp.tile([PT, NT * D], f32)
            nc.gpsimd.dma_start(
                qall[:].rearrange("p (t d) -> p t d", t=NT),
                q_h[:].rearrange("(t p) d -> p t d", p=PT),
            )

            def emit_load(t):
                rows = slice(t * PT, (t + 1) * PT)
                kt = kp.tile([PT, L * D], bf16, tag="keys")
                nc.gpsimd.dma_start(kt[:], k_h[rows].rearrange("b l d -> b (l d)"))
                kts[t] = kt

            def emit_dga(t):
                # v = q * W; duplicate into bf16 pairs; build 32-wide diag blocks
                vt = sp.tile([PT, D], f32, tag="v")
                nc.vector.tensor_mul(vt[:], qall[:, t * D : (t + 1) * D], wb[:])
                v2p = sp.tile([PT, 2 * D], bf16, tag="v2p")
                v2v = v2p[:].rearrange("p (d two) -> p d two", two=2)
                nc.vector.tensor_copy(v2v[:, :, 0], vt[:])
                nc.vector.tensor_copy(v2v[:, :, 1], vt[:])
                if dd:
                    # bf16 copy of v tail dims for the DVE partial scoring
                    vb = sp.tile([PT, dd], bf16, tag="vb")
                    nc.vector.tensor_copy(vb[:], vt[:, pd:])
                    vbs[t] = vb
                dga = dgp.tile([PT, pd * EW], bf16, tag="dg")
                nc.vector.tensor_mul(
                    dga[:].rearrange("p (d j2 two) -> p d j2 two", d=pd, two=2),
                    eye_t[:]
                    .rearrange("p (j2 two) -> p j2 two", two=2)
                    .unsqueeze(1)
                    .broadcast_to([PT, pd, EW // 2, 2]),
                    v2v[:, :pd].unsqueeze(2).broadcast_to([PT, pd, EW // 2, 2]),
                )
                dgas[t] = dga[:].rearrange("p (d j) -> p d j", d=pd)

            def emit_score_pe(t):
                psc = pp.tile([PT, L], f32, tag="sc")
                dg3 = dgas[t]
                k3 = kts[t][:].rearrange("p (l d) -> p l d", l=L)
                nblk = PT // EW
                for d in range(pd):
                    for i in range(nblk):
                        s = slice(EW * i, EW * i + EW)
                        nc.tensor.matmul(
                            psc[s, :], dg3[s, d, :], k3[s, :, d],
                            start=(d == 0), stop=(d == pd - 1),
                            tile_position=(EW * i, EW * i),
                        )
                pscs[t] = psc

            dve_scs = {}

            def emit_score_dve_partial(t):
                # DVE partial scoring over d >= pd (contiguous tail slices)
                if not dd:
                    return
                k3 = kts[t][:].rearrange("p (l d) -> p l d", l=L)
                inter = tp.tile([PT, L * dd], bf16, tag="inter")
                i3 = inter[:].rearrange("p (l e) -> p l e", l=L)
                nc.vector.tensor_mul(
                    i3, k3[:, :, pd:],
                    vbs[t][:].unsqueeze(1).broadcast_to([PT, L, dd]),
                )
                cur = i3
                n = dd
                lvl = 0
                while n > 2:
                    h = n // 2
                    tt = tp.tile([PT, L * h], bf16, tag=f"dt{lvl % 2}")
                    dst = tt[:].rearrange("p (l e) -> p l e", l=L)
                    nc.vector.tensor_add(dst, cur[:, :, 0:h], cur[:, :, h:n])
                    cur = dst
                    n = h
                    lvl += 1
                dve_sc = sp.tile([PT, L], f32, tag="dvesc")
                nc.vector.tensor_add(
                    dve_sc[:].unsqueeze(2), cur[:, :, 0:1], cur[:, :, 1:2]
                )
                dve_scs[t] = dve_sc

            def emit_score_add(t):
                # combine DVE partial with the PE partial (PSUM) -> f32 scores
                if dd:
                    sc = sp.tile([PT, L], f32, tag="scf")
                    nc.vector.tensor_add(sc[:], dve_scs[t][:], pscs[t][:])
                    scs[t] = sc
                else:
                    # all-PE scoring: tanh reads the PSUM scores directly
                    scs[t] = pscs[t]

            def emit_actsm(t):
                th = sp.tile([PT, L], f32, tag="th")
                nc.scalar.activation(th[:], scs[t][:], AF.Tanh)
                S = sp.tile([PT, 1], f32, tag="S")
                pe = sp.tile([PT, 2 * L], bf16, tag="pe")
                p3 = pe[:].rearrange("p (l two) -> p l two", two=2)
                nc.scalar.activation(p3[:, :, 0], th[:], AF.Exp, accum_out=S[:])
                nc.scalar.activation(p3[:, :, 1], th[:], AF.Exp)
                pexps[t] = pe
                sinv = sp.tile([PT, 1], f32, tag="sinv")
                nc.vector.reciprocal(sinv[:], S[:])
                sinvs[t] = sinv

            def emit_pool(t):
                kt = kts[t]
                wk = wkp.tile([PT, L * D], bf16, tag="wk")
                w3 = wk[:].rearrange("p (l d) -> p l d", l=L)
                p3 = pexps[t][:].rearrange("p (l two) -> p l two", two=2)
                nc.vector.tensor_mul(
                    wk[:].rearrange("p (l d2 two) -> p l d2 two", l=L, two=2),
                    kt[:].rearrange("p (l d2 two) -> p l d2 two", l=L, two=2),
                    p3.unsqueeze(2).broadcast_to([PT, L, D // 2, 2]),
                )
                ou = sp.tile([PT, D], f32, tag="ou")
                cur = w3
                n = L
                lvl = 0
                while n > 1:
                    h, odd = n // 2, n % 2
                    if h + odd == 1:
                        nc.vector.tensor_add(
                            ou[:].unsqueeze(1), cur[:, 0:1, :], cur[:, 1:2, :]
                        )
                        break
                    tt = tp.tile([PT, (h + odd) * D], bf16, tag=f"lt{lvl % 2}")
                    dst = tt[:].rearrange("p (n i) -> p n i", n=h + odd)
                    nc.vector.tensor_add(
                        dst[:, 0:h, :], cur[:, 0:h, :], cur[:, h : 2 * h, :]
                    )
                    if odd:
                        nc.vector.tensor_copy(
                            dst[:, h : h + 1, :], cur[:, 2 * h : n, :]
                        )
                    cur = dst
                    n = h + odd
                    lvl += 1
                of = sp.tile([PT, D], f32, tag="of")
                nc.vector.tensor_scalar_mul(of[:], ou[:], sinvs[t][:])
                rows = slice(t * PT, (t + 1) * PT)
                nc.sync.dma_start(o_h[rows, :], of[:])
                del kts[t], dgas[t], pscs[t], pexps[t], sinvs[t], scs[t]
                vbs.pop(t, None)
                dve_scs.pop(t, None)

            # ---- software pipeline ----
            # DVE order per steady iter: dga(t+1), add(t), partial(t+1) [fills
            # the ACT tanh/exp window], pool(t). dga(t+1) precedes add(t) so
            # PE(t+1) can start the moment PE(t) finishes.
            for t in range(min(PF, NT)):
                emit_load(t)
            if cfg.get("gsprobe"):
                # GPSIMD tensor_add rate probe: 6400 bf16 elems, off the
                # critical path (scratch output, input = first keys tile)
                gsx = cp.tile([PT, 6400], bf16)
                with nc.named_scope("gsprobe_add6400"):
                    nc.gpsimd.tensor_add(
                        gsx[:], kts[0][:, 0:6400], kts[0][:, 6400:12800]
                    )
            emit_dga(0)
            for t in range(NT):
                emit_score_pe(t)
                emit_score_dve_partial(t)
                emit_score_add(t)
                emit_actsm(t)
                if t + 1 < NT:
                    emit_dga(t + 1)
                if t + PF < NT:
                    emit_load(t + PF)
                emit_pool(t)

    nc.compile()
    return nc


def _get_nc_v2(cfg=None):
    cfg = dict(V2_CFG if cfg is None else cfg)
    key = ("v2",) + tuple(sorted(cfg.items()))
    if key not in _cache:
        _cache[key] = _build_v2(tuple(sorted(cfg.items())))
    return _cache[key]


def run_sharded_v2(query, keys, W, trace=False, cfg=None):
    cfg = dict(V2_CFG if cfg is None else cfg)
    query = np.ascontiguousarray(query, dtype=np.float32)
    keys = np.ascontiguousarray(keys, dtype=np.float32)
    W = np.ascontiguousarray(W, dtype=np.float32)
    nc = _get_nc_v2(cfg)
    w_b = np.ascontiguousarray(np.broadcast_to(W.reshape(1, D), (PT, D)))
    import ml_dtypes

    ew = cfg.get("ew", 32)
    e = np.zeros((PT, ew), dtype=ml_dtypes.bfloat16)
    e[np.arange(PT), np.arange(PT) % ew] = 1
    in_maps = [
        {
            "query": query[i * BC : (i + 1) * BC],
            "keys": keys[i * BC : (i + 1) * BC],
            "W": w_b,
            "eye": e,
        }
        for i in range(NCORES)
    ]
    res = run_bass_kernel_spmd(nc, in_maps, core_ids=list(range(NCORES)), trace=trace)
    out = np.concatenate([res.results[i]["out"] for i in range(NCORES)], axis=0)
    return out, res


# ---------------------------------------------------------------------------
# v3: ACT-transposed keys -> contiguous-rhs PE scoring, l-chunked pipeline
# ---------------------------------------------------------------------------
#
# Per 128-row tile (l split into CH chunks):
#   load(i):  keys chunk -> SBUF, [l, d] layout (SWDGE f32->bf16 cast, or
#             HWDGE f32, per cfg)
#   trans(i): ACT strided copy -> kT [d, l] bf16 (cast folded in for f32)
#   score(i): PE diag-matmuls, rhs = kT[:, d, :] now CONTIGUOUS (the [l, d]
#             layout's strided rhs ran at ~6 cyc/col and made PE the
#             bottleneck at 21.6us/tile)
#   tanh/exp(i) on ACT (exp unnormalized; accum_out -> chunk denominator)
#   pool(i):  DVE wk = kT * e broadcast over d; halving tree over l -> ou_c
#   fin(t):   S = sum_c S_c; ou = sum_c ou_c; out = ou / S
# Chunking works because exp needs no denominator until the end, so each
# chunk flows through the whole pipeline independently -> short drain tail.

V3_CFG = dict(ew=128, ch=2, pf=4, load="swdge")


def _build_v3(cfg_items):
    cfg = dict(cfg_items)
    EW = cfg.get("ew", 128)
    CH = cfg.get("ch", 2)
    PFC = cfg.get("pf", 4)       # prefetch depth, in chunks
    LOAD = cfg.get("load", "swdge")
    assert L % CH == 0
    LC = L // CH
    NBLK = PT // EW
    NCH = NT * CH
    AF = mybir.ActivationFunctionType

    nc = bacc.Bacc("TRN2", target_bir_lowering=False, debug=False, num_devices=NCORES)
    q_h = nc.declare_dram_parameter("query", [BC, D], f32, isOutput=False)
    k_h = nc.declare_dram_parameter("keys", [BC, L, D], f32, isOutput=False)
    w_h = nc.declare_dram_parameter("W", [PT, D], f32, isOutput=False)
    e_h = nc.declare_dram_parameter("eye", [PT, EW], bf16, isOutput=False)
    o_h = nc.declare_dram_parameter("out", [BC, D], f32, isOutput=True)

    def chunk_load_kind(i):
        if LOAD == "mixed":
            return "swdge" if (i % 2 == 0) else "hwdge"
        return LOAD

    kr_bufs = (PFC + 2) if LOAD == "swdge" else 3

    with tile.TileContext(nc) as tc:
        with (
            tc.tile_pool(name="kraw", bufs=kr_bufs) as krp,
            tc.tile_pool(name="kt", bufs=3) as ktp,
            tc.tile_pool(name="wk", bufs=2) as wkp,
            tc.tile_pool(name="tree", bufs=2) as tp,
            tc.tile_pool(name="small", bufs=3) as sp,
            tc.tile_pool(name="diag", bufs=2) as dgp,
            tc.tile_pool(name="psum", bufs=2, space="PSUM") as pp,
            tc.tile_pool(name="const", bufs=1) as cp,
        ):
            eye0 = cp.tile([PT, EW], bf16)
            nc.gpsimd.dma_start(eye0[:], e_h[:])
            eye_t = cp.tile([PT, EW], bf16)
            nc.vector.tensor_copy(eye_t[:], eye0[:])
            wb0 = cp.tile([PT, D], f32)
            nc.gpsimd.dma_start(wb0[:], w_h[:])
            wb = cp.tile([PT, D], f32)
            nc.vector.tensor_copy(wb[:], wb0[:])
            qall = cp.tile([PT, NT * D], f32)
            nc.gpsimd.dma_start(
                qall[:].rearrange("p (t d) -> p t d", t=NT),
                q_h[:].rearrange("(t p) d -> p t d", p=PT),
            )

            kraws, kts, dgas, pscs, pes_ = {}, {}, {}, {}, {}
            Ss, ous, sinvs = {}, {}, {}

            def emit_load(i):
                t, c = divmod(i, CH)
                rows = slice(t * PT, (t + 1) * PT)
                lsl = slice(c * LC, (c + 1) * LC)
                src = k_h[rows, lsl].rearrange("b l d -> b (l d)")
                if chunk_load_kind(i) == "swdge":
                    kr = krp.tile([PT, LC * D], bf16, tag="krb")
                    nc.gpsimd.dma_start(kr[:], src)
                else:
                    kr = krp.tile([PT, LC * D], f32, tag="krf")
                    nc.sync.dma_start(kr[:], src)
                kraws[i] = kr

            def emit_trans(i):
                # ACT: [l, d] -> [d, l] bf16 (strided read, contiguous write)
                kt = ktp.tile([PT, D * LC], bf16, tag="kt")
                nc.scalar.activation(
                    kt[:].rearrange("p (d l) -> p d l", d=D),
                    kraws[i][:].rearrange("p (l d) -> p d l", l=LC),
                    AF.Copy,
                )
                kts[i] = kt
                del kraws[i]

            def emit_dga(t):
                vt = sp.tile([PT, D], f32, tag="v")
                nc.vector.tensor_mul(vt[:], qall[:, t * D : (t + 1) * D], wb[:])
                v2p = sp.tile([PT, 2 * D], bf16, tag="v2p")
                v2v = v2p[:].rearrange("p (d two) -> p d two", two=2)
                nc.vector.tensor_copy(v2v[:, :, 0], vt[:])
                nc.vector.tensor_copy(v2v[:, :, 1], vt[:])
                dga = dgp.tile([PT, D * EW], bf16, tag="dg")
                nc.vector.tensor_mul(
                    dga[:].rearrange("p (d j2 two) -> p d j2 two", d=D, two=2),
                    eye_t[:]
                    .rearrange("p (j2 two) -> p j2 two", two=2)
                    .unsqueeze(1)
                    .broadcast_to([PT, D, EW // 2, 2]),
                    v2v.unsqueeze(2).broadcast_to([PT, D, EW // 2, 2]),
                )
                dgas[t] = dga[:].rearrange("p (d j) -> p d j", d=D)

            def emit_score(i):
                t, c = divmod(i, CH)
                psc = pp.tile([PT, LC], f32, tag=f"sc{c}")
                kt3 = kts[i][:].rearrange("p (d l) -> p d l", d=D)
                dg3 = dgas[t]
                for d in range(D):
                    for b in range(NBLK):
                        s = slice(EW * b, EW * (b + 1))
                        nc.tensor.matmul(
                            psc[s, :], dg3[s, d, :], kt3[s, d, :],
                            start=(d == 0), stop=(d == D - 1),
                            tile_position=(EW * b, EW * b),
                        )
                pscs[i] = psc

            def emit_actsm(i):
                t, c = divmod(i, CH)
                th = sp.tile([PT, LC], f32, tag="th")
                nc.scalar.activation(th[:], pscs[i][:], AF.Tanh)
                S = sp.tile([PT, 1], f32, tag=f"S{c}")
                pe = sp.tile([PT, LC], bf16, tag=f"pe{c}")
                nc.scalar.activation(pe[:], th[:], AF.Exp, accum_out=S[:])
                pes_[i] = pe
                Ss[i] = S
                del pscs[i]

            def emit_pool(i):
                t, c = divmod(i, CH)
                kt3 = kts[i][:].rearrange("p (d l) -> p d l", d=D)
                wk = wkp.tile([PT, D * LC], bf16, tag="wk")
                w3 = wk[:].rearrange("p (d l) -> p d l", d=D)
                nc.vector.tensor_mul(
                    w3, kt3, pes_[i][:].unsqueeze(1).broadcast_to([PT, D, LC])
                )
                # halving tree over l (innermost, contiguous slices)
                ou = sp.tile([PT, D], f32, tag=f"ou{c}")
                cur = w3
                n = LC
                lvl = 0
                while n > 1:
                    h, odd = n // 2, n % 2
                    if h + odd == 1:
                        nc.vector.tensor_add(
                            ou[:].unsqueeze(2), cur[:, :, 0:1], cur[:, :, 1:2]
                        )
                        break
                    tt = tp.tile([PT, D * (h + odd)], bf16, tag=f"t{lvl % 2}")
                    dst = tt[:].rearrange("p (o i) -> p o i", o=D)
                    nc.vector.tensor_add(
                        dst[:, :, 0:h], cur[:, :, 0:h], cur[:, :, h : 2 * h]
                    )
                    if odd:
                        nc.vector.tensor_copy(
                            dst[:, :, h : h + 1], cur[:, :, 2 * h : n]
                        )
                    cur = dst
                    n = h + odd
                    lvl += 1
                ous[i] = ou
                del kts[i], pes_[i]

            def emit_fin(t):
                i0 = t * CH
                if CH > 1:
                    St = sp.tile([PT, 1], f32, tag="St")
                    nc.vector.tensor_add(St[:], Ss[i0][:], Ss[i0 + 1][:])
                    for c in range(2, CH):
                        nc.vector.tensor_add(St[:], St[:], Ss[i0 + c][:])
                    out_u = sp.tile([PT, D], f32, tag="outu")
                    nc.vector.tensor_add(out_u[:], ous[i0][:], ous[i0 + 1][:])
                    for c in range(2, CH):
                        nc.vector.tensor_add(out_u[:], out_u[:], ous[i0 + c][:])
                else:
                    St = Ss[i0]
                    out_u = ous[i0]
                sinv = sp.tile([PT, 1], f32, tag="sinv")
                nc.vector.reciprocal(sinv[:], St[:])
                of = sp.tile([PT, D], f32, tag="of")
                nc.vector.tensor_scalar_mul(of[:], out_u[:], sinv[:])
                rows = slice(t * PT, (t + 1) * PT)
                nc.scalar.dma_start(o_h[rows, :], of[:])
                for c in range(CH):
                    del Ss[i0 + c], ous[i0 + c]
                del dgas[t]

            # ---- software pipeline over chunks ----
            for i in range(min(PFC, NCH)):
                emit_load(i)
            emit_dga(0)
            emit_trans(0)
            for i in range(NCH):
                t, c = divmod(i, CH)
                if i + 1 < NCH:
                    emit_trans(i + 1)       # ACT: keep transpose a chunk ahead
                    if (i + 1) % CH == 0:
                        emit_dga(t + 1)     # DVE: diag for next tile
                emit_score(i)
                emit_actsm(i)
                if i + PFC < NCH:
                    emit_load(i + PFC)
                emit_pool(i)
                if c == CH - 1:
                    emit_fin(t)

    nc.compile()
    return nc


def _get_nc_v3(cfg=None):
    cfg = dict(V3_CFG if cfg is None else cfg)
    key = ("v3",) + tuple(sorted(cfg.items()))
    if key not in _cache:
        _cache[key] = _build_v3(tuple(sorted(cfg.items())))
    return _cache[key]


def run_sharded_v3(query, keys, W, trace=False, cfg=None):
    cfg = dict(V3_CFG if cfg is None else cfg)
    query = np.ascontiguousarray(query, dtype=np.float32)
    keys = np.ascontiguousarray(keys, dtype=np.float32)
    W = np.ascontiguousarray(W, dtype=np.float32)
    nc = _get_nc_v3(cfg)
    w_b = np.ascontiguousarray(np.broadcast_to(W.reshape(1, D), (PT, D)))
    import ml_dtypes

    ew = cfg.get("ew", 128)
    e = np.zeros((PT, ew), dtype=ml_dtypes.bfloat16)
    e[np.arange(PT), np.arange(PT) % ew] = 1
    in_maps = [
        {
            "query": query[i * BC : (i + 1) * BC],
            "keys": keys[i * BC : (i + 1) * BC],
            "W": w_b,
            "eye": e,
        }
        for i in range(NCORES)
    ]
    res = run_bass_kernel_spmd(nc, in_maps, core_ids=list(range(NCORES)), trace=trace)
    out = np.concatenate([res.results[i]["out"] for i in range(NCORES)], axis=0)
    return out, res


# ---------------------------------------------------------------------------
# v4: d-split scoring (strided-PE + partially-transposed contiguous-PE),
# pooling from the raw [l, d] tile, l-chunked pipeline.
# ---------------------------------------------------------------------------
#
# The 64 score dims are split: dims [0, cd) get ACT-transposed to [d, l] and
# scored with contiguous-rhs matmuls (fast PE, costs ACT time); dims
# [cd, 64) are scored straight from the raw [l, d] tile with strided-rhs
# matmuls (~4 cyc/col, costs PE time but nothing else). Both accumulate
# into one PSUM group. Pooling (exp-weighted sum over l) runs on DVE from
# the raw tile with the bf16-pair broadcast trick, unchanged from v2.

V4_CFG = dict(sd=32, cd=32, ewc=64, ch=2, pf=4, tailch=4, load="swdge")


def _build_v4(cfg_items):
    cfg = dict(cfg_items)
    SD = cfg.get("sd", 32)       # strided-PE dims (regular tiles)
    CD = cfg.get("cd", 32)       # contiguous-PE dims (ACT-transposed)
    EWC = cfg.get("ewc", 64)     # block width for contiguous diag matmuls
    CH = cfg.get("ch", 2)
    PFC = cfg.get("pf", 4)
    TAILCH = cfg.get("tailch", 4)  # chunks for the last tile (pure strided)
    assert SD + CD == D
    assert L % CH == 0 and L % TAILCH == 0
    EWS = 32
    EYEW = (EWC if CD else 0) + EWS
    AF = mybir.ActivationFunctionType

    # chunk descriptors: tiles 0..NT-2 use CH chunks with the sd/cd split;
    # the last tile uses TAILCH chunks, scored pure-strided, so the drain
    # tail after the final DMA is short.
    chunks = []
    for t in range(NT):
        nch = CH if t < NT - 1 else TAILCH
        lc = L // nch
        for c in range(nch):
            chunks.append({"t": t, "c": c, "nch": nch, "l0": c * lc, "lc": lc})
    NCHK = len(chunks)

    nc = bacc.Bacc("TRN2", target_bir_lowering=False, debug=False, num_devices=NCORES)
    # qw2: host-precomputed (query * W) duplicated into bf16 pairs
    q_h = nc.declare_dram_parameter("qw2", [BC, 2 * D], bf16, isOutput=False)
    k_h = nc.declare_dram_parameter("keys", [BC, L, D], f32, isOutput=False)
    e_h = nc.declare_dram_parameter("eye", [PT, EYEW], bf16, isOutput=False)
    o_h = nc.declare_dram_parameter("out", [BC, D], f32, isOutput=True)

    with tile.TileContext(nc) as tc:
        with (
            tc.tile_pool(name="kraw", bufs=PFC + 3) as krp,
            tc.tile_pool(name="kt", bufs=3) as ktp,
            tc.tile_pool(name="wk", bufs=2) as wkp,
            tc.tile_pool(name="tree", bufs=2) as tp,
            tc.tile_pool(name="small", bufs=3) as sp,
            tc.tile_pool(name="diag", bufs=2) as dgp,
            tc.tile_pool(name="psum", bufs=2, space="PSUM") as pp,
            tc.tile_pool(name="const", bufs=1) as cp,
        ):
            kraws, kts, dgcs, dgss, pscs, peds = {}, {}, {}, {}, {}, {}
            Ss, ous = {}, {}

            def emit_load(i):
                ck = chunks[i]
                rows = slice(ck["t"] * PT, (ck["t"] + 1) * PT)
                lsl = slice(ck["l0"], ck["l0"] + ck["lc"])
                src = k_h[rows, lsl].rearrange("b l d -> b (l d)")
                kr = krp.tile([PT, ck["lc"] * D], bf16, tag="krb")
                nc.gpsimd.dma_start(kr[:], src)
                kraws[i] = kr

            def emit_preamble():
                eye0 = cp.tile([PT, EYEW], bf16)
                nc.gpsimd.dma_start(eye0[:], e_h[:])
                eye_t = cp.tile([PT, EYEW], bf16)
                nc.vector.tensor_copy(eye_t[:], eye0[:])
                qall = cp.tile([PT, NT * 2 * D], bf16)
                nc.gpsimd.dma_start(
                    qall[:].rearrange("p (t d2) -> p t d2", t=NT),
                    q_h[:].rearrange("(t p) d2 -> p t d2", p=PT),
                )
                return eye_t, qall

            def emit_dga(t):
                # v-pairs for this tile, straight from the host-packed qall
                v2v = qall[:, t * 2 * D : (t + 1) * 2 * D].rearrange(
                    "p (d two) -> p d two", two=2
                )
                tail = t == NT - 1
                ncd = 0 if tail else CD
                nsd = D - ncd
                if ncd:
                    eyec = eye_t[:, 0:EWC]
                    dgc = dgp.tile([PT, ncd * EWC], bf16, tag="dgc")
                    nc.vector.tensor_mul(
                        dgc[:].rearrange("p (d j2 two) -> p d j2 two", d=ncd, two=2),
                        eyec.rearrange("p (j2 two) -> p j2 two", two=2)
                        .unsqueeze(1)
                        .broadcast_to([PT, ncd, EWC // 2, 2]),
                        v2v[:, 0:ncd].unsqueeze(2).broadcast_to([PT, ncd, EWC // 2, 2]),
                    )
                    dgcs[t] = dgc[:].rearrange("p (d j) -> p d j", d=ncd)
                eyes = eye_t[:, EWC if CD else 0 :]
                dgs = dgp.tile([PT, D * EWS], bf16, tag="dgs")
                nc.vector.tensor_mul(
                    dgs[:, 0 : nsd * EWS].rearrange(
                        "p (d j2 two) -> p d j2 two", d=nsd, two=2
                    ),
                    eyes.rearrange("p (j2 two) -> p j2 two", two=2)
                    .unsqueeze(1)
                    .broadcast_to([PT, nsd, EWS // 2, 2]),
                    v2v[:, ncd:D].unsqueeze(2).broadcast_to([PT, nsd, EWS // 2, 2]),
                )
                dgss[t] = dgs[:, 0 : nsd * EWS].rearrange("p (d j) -> p d j", d=nsd)

            def emit_trans(i):
                ck = chunks[i]
                if ck["t"] == NT - 1 or not CD:
                    return
                lc = ck["lc"]
                kt = ktp.tile([PT, CD * lc], bf16, tag="kt")
                nc.scalar.activation(
                    kt[:].rearrange("p (d l) -> p d l", d=CD),
                    kraws[i][:].rearrange("p (l d) -> p d l", l=lc)[:, 0:CD, :],
                    AF.Copy,
                )
                kts[i] = kt

            def emit_score(i):
                ck = chunks[i]
                t, c, lc = ck["t"], ck["c"], ck["lc"]
                tail = t == NT - 1
                ncd = 0 if tail else CD
                nsd = D - ncd
                pscf = pp.tile([PT, L // CH], f32, tag=f"sc{c % 2}")
                psc = pscf[:, 0:lc]
                total = D
                done = 0
                if ncd:
                    kt3 = kts[i][:].rearrange("p (d l) -> p d l", d=ncd)
                    dg3 = dgcs[t]
                    for d in range(ncd):
                        done += 1
                        for b in range(PT // EWC):
                            s = slice(EWC * b, EWC * (b + 1))
                            nc.tensor.matmul(
                                psc[s, :], dg3[s, d, :], kt3[s, d, :],
                                start=(d == 0), stop=(done == total),
                                tile_position=(EWC * b, EWC * b),
                                skip_group_check=True,
                            )
                k3 = kraws[i][:].rearrange("p (l d) -> p l d", l=lc)
                dg3 = dgss[t]
                for d in range(nsd):
                    done += 1
                    for b in range(PT // EWS):
                        s = slice(EWS * b, EWS * (b + 1))
                        nc.tensor.matmul(
                            psc[s, :], dg3[s, d, :], k3[s, :, ncd + d],
                            start=(ncd == 0 and d == 0), stop=(done == total),
                            tile_position=(EWS * b, EWS * b),
                            skip_group_check=True,
                        )
                pscs[i] = psc
                kts.pop(i, None)

            def emit_actsm(i):
                ck = chunks[i]
                lc, c = ck["lc"], ck["c"]
                th = sp.tile([PT, L // CH], f32, tag="th")
                nc.scalar.activation(th[:, 0:lc], pscs[i][:], AF.Tanh)
                S = sp.tile([PT, 1], f32, tag=f"S{c % 2}")
                ped = sp.tile([PT, 2 * (L // CH)], bf16, tag=f"pe{c % 2}")
                p3 = ped[:, 0 : 2 * lc].rearrange("p (l two) -> p l two", two=2)
                nc.scalar.activation(p3[:, :, 0], th[:, 0:lc], AF.Exp, accum_out=S[:])
                nc.scalar.activation(p3[:, :, 1], th[:, 0:lc], AF.Exp)
                peds[i] = ped
                Ss[i] = S
                del pscs[i]

            def emit_pool(i):
                ck = chunks[i]
                t, c, lc = ck["t"], ck["c"], ck["lc"]
                kr = kraws[i]
                wk = wkp.tile([PT, (L // CH) * D], bf16, tag="wk")
                w3 = wk[:, 0 : lc * D].rearrange("p (l d) -> p l d", l=lc)
                p3 = peds[i][:, 0 : 2 * lc].rearrange("p (l two) -> p l two", two=2)
                nc.vector.tensor_mul(
                    wk[:, 0 : lc * D].rearrange(
                        "p (l d2 two) -> p l d2 two", l=lc, two=2
                    ),
                    kr[:].rearrange("p (l d2 two) -> p l d2 two", l=lc, two=2),
                    p3.unsqueeze(2).broadcast_to([PT, lc, D // 2, 2]),
                )
                # halving tree over l; below n=8, one strided reduce finishes it
                ou = sp.tile([PT, D], f32, tag=f"ou{c % 2}")
                cur = w3
                n = lc
                lvl = 0
                while n > 7:
                    h, odd = n // 2, n % 2
                    tt = tp.tile([PT, (L // CH) * D // 2], bf16, tag=f"t{lvl % 2}")
                    dst = tt[:, 0 : (h + odd) * D].rearrange(
                        "p (n i) -> p n i", n=h + odd
                    )
                    nc.vector.tensor_add(
                        dst[:, 0:h, :], cur[:, 0:h, :], cur[:, h : 2 * h, :]
                    )
                    if odd:
                        nc.vector.tensor_copy(
                            dst[:, h : h + 1, :], cur[:, 2 * h : n, :]
                        )
                    cur = dst
                    n = h + odd
                    lvl += 1
                nc.vector.reduce_sum(
                    ou[:], cur[:, 0:n, :].rearrange("p n d -> p d n"),
                    axis=mybir.AxisListType.X,
                )
                ous[i] = ou
                del kraws[i], peds[i]

            def emit_fin(t):
                idxs = [i for i, ck in enumerate(chunks) if ck["t"] == t]
                i0 = idxs[0]
                if len(idxs) > 1:
                    St = sp.tile([PT, 1], f32, tag="St")
                    nc.scalar.add(St[:], Ss[i0][:], Ss[idxs[1]][:])
                    for j in idxs[2:]:
                        nc.scalar.add(St[:], St[:], Ss[j][:])
                    out_u = sp.tile([PT, D], f32, tag="outu")
                    nc.vector.tensor_add(out_u[:], ous[i0][:], ous[idxs[1]][:])
                    for j in idxs[2:]:
                        nc.vector.tensor_add(out_u[:], out_u[:], ous[j][:])
                else:
                    St = Ss[i0]
                    out_u = ous[i0]
                sinv = sp.tile([PT, 1], f32, tag="sinv")
                nc.vector.reciprocal(sinv[:], St[:])
                of = sp.tile([PT, D], f32, tag="of")
                nc.scalar.mul(of[:], out_u[:], sinv[:])
                rows = slice(t * PT, (t + 1) * PT)
                nc.scalar.dma_start(o_h[rows, :], of[:])
                for j in idxs:
                    del Ss[j], ous[j]
                dgcs.pop(t, None)
                dgss.pop(t, None)

            # ---- pipeline: keys DMA first, then consts, then steady state
            for i in range(min(PFC, NCHK)):
                emit_load(i)
            eye_t, qall = emit_preamble()
            emit_dga(0)
            emit_trans(0)
            for i in range(NCHK):
                ck = chunks[i]
                if i + 1 < NCHK:
                    emit_trans(i + 1)
                    if chunks[i + 1]["t"] != ck["t"]:
                        emit_dga(chunks[i + 1]["t"])
                emit_score(i)
                emit_actsm(i)
                if i + PFC < NCHK:
                    emit_load(i + PFC)
                emit_pool(i)
                if ck["c"] == ck["nch"] - 1:
                    emit_fin(ck["t"])

    nc.compile()
    return nc


def _get_nc_v4(cfg=None):
    cfg = dict(V4_CFG if cfg is None else cfg)
    key = ("v4",) + tuple(sorted(cfg.items()))
    if key not in _cache:
        _cache[key] = _build_v4(tuple(sorted(cfg.items())))
    return _cache[key]


def run_sharded_v4(query, keys, W, trace=False, cfg=None):
    cfg = dict(V4_CFG if cfg is None else cfg)
    query = np.ascontiguousarray(query, dtype=np.float32)
    keys = np.ascontiguousarray(keys, dtype=np.float32)
    W = np.ascontiguousarray(W, dtype=np.float32)
    nc = _get_nc_v4(cfg)
    import ml_dtypes

    # host-side: qw = query * W, duplicated into adjacent bf16 pairs
    qw = (query * W.reshape(1, D)).astype(ml_dtypes.bfloat16)
    qw2 = np.ascontiguousarray(np.repeat(qw, 2, axis=1))
    cd, ewc = cfg.get("cd", 32), cfg.get("ewc", 64)
    cols = []
    if cd:
        ec = np.zeros((PT, ewc), dtype=ml_dtypes.bfloat16)
        ec[np.arange(PT), np.arange(PT) % ewc] = 1
        cols.append(ec)
    es = np.zeros((PT, 32), dtype=ml_dtypes.bfloat16)
    es[np.arange(PT), np.arange(PT) % 32] = 1
    cols.append(es)
    e = np.ascontiguousarray(np.concatenate(cols, axis=1))
    in_maps = [
        {
            "qw2": qw2[i * BC : (i + 1) * BC],
            "keys": keys[i * BC : (i + 1) * BC],
            "eye": e,
        }
        for i in range(NCORES)
    ]
    res = run_bass_kernel_spmd(nc, in_maps, core_ids=list(range(NCORES)), trace=trace)
    out = np.concatenate([res.results[i]["out"] for i in range(NCORES)], axis=0)
    return out, res


def kernel(query, keys, W):
    if _run_state.get("use_v4", True):
        try:
            out, _ = run_sharded_v4(query, keys, W, trace=False)
            if not _run_state.get("checked_v4"):
                _run_state["checked_v4"] = True
                if _spot_check(out, query, keys, W) > 2e-2:
                    raise RuntimeError("v4 accuracy check failed")
            return out
        except Exception:
            _run_state["use_v4"] = False
            _run_state["checked_v4"] = False
    if _run_state.get("use_v2", True):
        try:
            out, _ = run_sharded_v2(query, keys, W, trace=False)
            if not _run_state["checked"]:
                _run_state["checked"] = True
                if _spot_check(out, query, keys, W) > 2e-2:
                    raise RuntimeError("v2 accuracy check failed")
            return out
        except Exception:
            _run_state["use_v2"] = False
            _run_state["checked"] = False
    var = _run_state["variant"]
    try:
        out, _ = run_sharded(query, keys, W, trace=False, variant=var)
        if var != SAFE_VARIANT and not _run_state["checked"]:
            _run_state["checked"] = True
            if _spot_check(out, query, keys, W) > 2e-2:
                raise RuntimeError("fast-variant accuracy check failed")
    except Exception:
        if var == SAFE_VARIANT:
            raise
        _run_state["variant"] = SAFE_VARIANT
        out, _ = run_sharded(query, keys, W, trace=False, variant=SAFE_VARIANT)
    return out

